# revision 10
# baseline (speedup 1.0000x reference)
"""Trainium2 Bass kernel v2: 21 depthwise Gaussian blurs + channel concat.

Problem: x (8, 3, 512, 512) f32 -> out (8, 66, 512, 512) f32 where
out = concat([x, blur_0(x), ..., blur_20(x)], axis=1) and blur_i is a
depthwise 2D Gaussian conv (reflect padding, kernel sizes 3..21).

Strategy (pure data parallel, 1 image per core across 8 cores):
  Each Gaussian is separable: conv2d(outer(g,g)) = conv_H(g) then conv_W(g).
  Each 1D conv (with reflect pad folded in) is a banded 512x512 matrix M.
  Per channel-image X (512x512):   Y_i = M_i @ X @ M_i^T
  Pass 1 computes Z^T = (M X)^T via out = lhsT.T @ rhs with lhsT = X-block
  (stationary) and rhs = band slabs of M^T (moving, ~130-148 cols per
  128-row block).  Pass 2 repeats the same structure on Z^T, which
  transposes back, yielding Y in natural [h, w] layout.

v2 over baseline:
  - output stored as f16 on device (33 MB vs 66 MB of HBM writes per
    core); host upcasts to f32.  Per-channel rel err stays ~4e-3
    (bf16 matmuls dominate), well under the 2e-2 gate.
  - passthrough channels (out[:, :3] = x) are filled on the host; the
    device only computes + writes the 63 blur channels.
  - x is cast to bf16 on the host; device loads bf16 directly (no
    GpSimd cast pass, half the input DMA).
  - PSUM evacuation copies rotate across ACT/DVE/Pool (5:4:3) instead
    of ACT/DVE only.
"""

import numpy as np
import ml_dtypes

B, C, H, W = 8, 3, 512, 512
N = 512
P = 128
NBLK = N // P  # 4

NUM_KERNELS = 21
MAX_KSIZE = 21
INIT_KSIZE = 3
_INCREMENT = (MAX_KSIZE - INIT_KSIZE) / (NUM_KERNELS - 2)
KSIZES = [
    min(MAX_KSIZE, int(INIT_KSIZE + i * _INCREMENT // 2 * 2))
    for i in range(NUM_KERNELS)
]
SIGMAS = np.linspace(0.5, 1.0, NUM_KERNELS)

import os as _os

TRACE = False  # set True (from a driver) to capture an NTFF profile
MERGE_LDW = _os.environ.get("KMERGE_LDW", "0") == "1"  # fold LDWEIGHTS into matmuls
LDW_OPT = _os.environ.get("KLDW_OPT", "0") == "1"  # --enable-ldw-opt=true (needs MERGE_LDW)
LAST_RESULTS = {}  # driver-inspectable: exec_time_ns etc.


def _gauss1d(k, sigma):
    # Matches reference _gauss_kernel numerics: float32 arange, float64 sigma
    # promotes the math to float64; normalized to sum 1.
    x = np.arange(k, dtype=np.float32)
    g = np.exp(-((x - k // 2) ** 2) / (2.0 * sigma**2))
    return g / g.sum()


def _conv_matrix(g, n=N):
    """Banded matrix M (float64) s.t. y = M @ x computes the reflect-padded
    1D convolution with taps g."""
    k = len(g)
    p = (k - 1) // 2
    M = np.zeros((n, n), np.float64)
    for r in range(n):
        for t in range(k):
            c = r + t - p
            if c < 0:
                c = -c
            elif c >= n:
                c = 2 * (n - 1) - c
            M[r, c] += g[t]
    return M


def _slab_geometry():
    """Per (kernel, block) slab column ranges in M^T, plus pack offsets."""
    geo = []  # [i][b] = (clo, chi, off)
    off = 0
    for i in range(NUM_KERNELS):
        p = (KSIZES[i] - 1) // 2
        row = []
        for b in range(NBLK):
            clo = max(0, P * b - p)
            chi = min(N, P * b + P + p)
            row.append((clo, chi, off))
            off += chi - clo
        geo.append(row)
    return geo, off


def _build_wpack():
    geo, totalw = _slab_geometry()
    wpack = np.zeros((P, totalw), ml_dtypes.bfloat16)
    for i in range(NUM_KERNELS):
        MT = _conv_matrix(_gauss1d(KSIZES[i], SIGMAS[i])).T
        for b in range(NBLK):
            clo, chi, off = geo[i][b]
            wpack[:, off : off + (chi - clo)] = MT[P * b : P * b + P, clo:chi].astype(
                ml_dtypes.bfloat16
            )
    return geo, totalw, wpack


_GEO, _TOTALW, _WPACK = None, None, None
_NC = None


def _consts():
    global _GEO, _TOTALW, _WPACK
    if _WPACK is None:
        _GEO, _TOTALW, _WPACK = _build_wpack()
    return _GEO, _TOTALW, _WPACK


def _build_nc():
    import concourse.bacc as bacc
    import concourse.mybir as mybir
    from concourse.tile import TileContext

    geo, totalw, _ = _consts()
    bf16 = mybir.dt.bfloat16
    f16 = mybir.dt.float16
    f32 = mybir.dt.float32

    nc = bacc.Bacc("TRN2", target_bir_lowering=False)
    # Partition-major layouts: x_perm[c, p, b, :] = x[c, 128*b + p, :] and
    # y_perm[c, p, b, :] = y[c, 128*b + p, :].  Each DMA then moves one
    # 4 KiB contiguous DRAM run per partition (128 descriptors) instead of
    # four 1 KiB runs (512 descriptors) -- descriptor generation on the
    # issuing sequencer was a serial bottleneck.
    x = nc.dram_tensor("x", [C, P, NBLK * N], bf16, kind="ExternalInput")
    w = nc.dram_tensor("w", [P, totalw], bf16, kind="ExternalInput")
    y = nc.dram_tensor("y", [C * NUM_KERNELS, P, NBLK * N], f16, kind="ExternalOutput")

    # PSUM evacuation: GPSIMD cannot touch PSUM on TRN2, so only ACT
    # (measured ~1.04us per [128,1024] copy) and DVE (~1.15us) qualify.
    # Pass-1 (z) evacuations gate pass 2 on the PE, so the two z copies of
    # an item always go to OPPOSITE engines (concurrent, minimal latency).
    # y copies are distributed to balance total busy time (~69:57).
    ny = 0

    def evac_z(nc_, dst, src, half):
        if half == 0:
            nc_.scalar.copy(dst, src)
        else:
            nc_.vector.tensor_copy(dst, src)

    def evac_y(nc_, dst, src):
        nonlocal ny
        ny += 1
        # 11:9 ACT:DVE over a 20-cycle pattern ~ balances engine busy
        if ny % 20 < 11:
            nc_.scalar.copy(dst, src)
        else:
            nc_.vector.tensor_copy(dst, src)

    with TileContext(nc) as tc:
        with (
            tc.tile_pool(name="wsb", bufs=1) as wpool,
            tc.tile_pool(name="xsb", bufs=12) as xpool,
            tc.tile_pool(name="zt", bufs=8) as ztpool,
            tc.tile_pool(name="yo", bufs=6) as ypool,
            tc.tile_pool(name="ps1", bufs=2, space="PSUM") as ps1,
            tc.tile_pool(name="ps2", bufs=2, space="PSUM") as ps2,
        ):
            # One [128, 2048] load per channel; column range [512j, 512j+512)
            # of the tile is x rows [128j, 128j+128).
            xch = {}
            for ci in range(C):
                t = xpool.tile([P, NBLK * N], bf16, tag="x")
                nc.sync.dma_start(t[:], x[ci])
                xch[ci] = t

            # band slabs, chunked so early kernels start before the full load
            wsb = wpool.tile([P, totalw], bf16)
            bounds = [geo[i][0][2] for i in range(0, NUM_KERNELS, 3)] + [totalw]
            for a, b in zip(bounds[:-1], bounds[1:]):
                nc.sync.dma_start(wsb[:, a:b], w[:, a:b])

            def emit_pass1(ci, i):
                # ---- pass 1: Z^T[wb] = sum_j X[j,wb]^T @ slab(i,j) ----
                # two wb blocks share one 2-bank PSUM tile -> one big copy
                xt = xch[ci]
                zt = []
                for wb2 in range(NBLK // 2):
                    psz = ps1.tile([P, 2 * N], f32, tag="psz")
                    for half in range(2):
                        wb = 2 * wb2 + half
                        for j in range(NBLK):
                            clo, chi, off = geo[i][j]
                            nc.tensor.matmul(
                                psz[:, half * N + clo : half * N + chi],
                                xt[:, N * j + P * wb : N * j + P * wb + P],
                                wsb[:, off : off + (chi - clo)],
                                start=(j == 0),
                                stop=(j == NBLK - 1),
                            )
                    zt2 = ztpool.tile([P, 2 * N], bf16, tag="zt")
                    evac_z(nc, zt2[:], psz[:], wb2)
                    zt.append(zt2)
                return zt

            def emit_pass2(ci, i, zt):
                # ---- pass 2: Y[hb] = sum_wb Z^T[wb,hb]^T @ slab(i,wb) ----
                def ztap(wb, hb):
                    # Z^T[wb] block, columns for h-block hb
                    return zt[wb // 2][:, (wb % 2) * N + P * hb : (wb % 2) * N + P * hb + P]

                cout = C * i + ci
                yo = ypool.tile([P, 4 * N], f16, tag="yo")
                for hb2 in range(NBLK // 2):
                    psy = ps2.tile([P, 2 * N], f32, tag="psy")
                    for half in range(2):
                        hb = 2 * hb2 + half
                        for wb in range(NBLK):
                            clo, chi, off = geo[i][wb]
                            nc.tensor.matmul(
                                psy[:, half * N + clo : half * N + chi],
                                ztap(wb, hb),
                                wsb[:, off : off + (chi - clo)],
                                start=(wb == 0),
                                stop=(wb == NBLK - 1),
                            )
                    evac_y(nc, yo[:, hb2 * 2 * N : (hb2 + 1) * 2 * N], psy[:])
                nc.sync.dma_start(y[cout], yo[:])

            # Software-pipeline by one stage: emit pass 1 of item k+1 before
            # pass 2 of item k, so the in-order PE has independent matmuls to
            # run while item k's PSUM->SBUF evacuations are in flight.
            items = [(ci, i) for ci in range(C) for i in range(NUM_KERNELS)]
            zt_prev = emit_pass1(*items[0])
            for idx, (ci, i) in enumerate(items):
                zt_next = (
                    emit_pass1(*items[idx + 1]) if idx + 1 < len(items) else None
                )
                emit_pass2(ci, i, zt_prev)
                zt_prev = zt_next

    nc.finalize()
    if MERGE_LDW:
        _merge_ldweights(nc)
    return nc


def _merge_ldweights(nc):
    """Fold the Tile-emitted standalone InstLdweights back into their
    InstMatmult (self-loading form) so walrus's LDW optimization (fast
    weight load) can apply.  LDWs carrying sync waits are replaced by an
    InstEventSemaphore stub at the same position to preserve ordering."""
    import concourse.mybir as mybir

    ev = 0
    for blk in nc.m.functions[0].blocks:
        insts = blk.instructions
        new = []
        changed = False
        for ins in insts:
            tn = type(ins).__name__
            if tn == "InstLdweights":
                changed = True
                si = ins.sync_info
                if si is not None and (si.on_wait or si.on_update):
                    e = mybir.InstEventSemaphore(
                        name=f"ldw_ev_{ev}", ins=[], outs=[]
                    )
                    ev += 1
                    e.engine = ins.engine
                    e.sync_info = si
                    new.append(e)
                continue
            if tn == "InstMatmult":
                ins.ldweights = True
            new.append(ins)
        if changed:
            del insts[:]
            insts.extend(new)


def _patch_ldw_opt():
    import concourse.bass_utils as bass_utils

    if getattr(bass_utils, "_ldw_opt_patched", False):
        return
    orig = bass_utils.run_command

    def patched(argv, **kw):
        argv = [
            "--enable-ldw-opt=true" if a == "--enable-ldw-opt=false" else a
            for a in argv
        ]
        return orig(argv, **kw)

    bass_utils.run_command = patched
    bass_utils._ldw_opt_patched = True


def _get_nc():
    global _NC
    if _NC is None:
        _NC = _build_nc()
    return _NC


def _install_trace_hook():
    """Best-effort NTFF profiling hook for axon (used when TRACE=True)."""
    import sys
    import types

    if "antenv.axon_hooks" in sys.modules:
        return
    m = types.ModuleType("antenv.axon_hooks")
    m._hook = None
    m.set_axon_ntff_profile_hook = lambda h: setattr(m, "_hook", h)
    m.get_axon_ntff_profile_hook = lambda: m._hook
    sys.modules["antenv.axon_hooks"] = m
    try:
        import antenv

        antenv.axon_hooks = m
        from trn_agent_boot.trn_boot import _ntff_profile_via_ctypes

        m._hook = _ntff_profile_via_ctypes("/opt/axon/libaxon_pjrt.so")
    except Exception:
        pass


def kernel(x):
    import concourse.bass_utils as bass_utils

    if LDW_OPT:
        _patch_ldw_opt()
    x = np.asarray(x, dtype=np.float32)
    assert x.shape == (B, C, H, W), x.shape
    _, _, wpack = _consts()
    nc = _get_nc()

    # partition-major device layout: x_perm[c, p, b*512+w] = x[c, 128b+p, w]
    x_bf = np.ascontiguousarray(
        x.astype(ml_dtypes.bfloat16)
        .reshape(B, C, NBLK, P, W)
        .transpose(0, 1, 3, 2, 4)
        .reshape(B, C, P, NBLK * W)
    )
    in_maps = [{"x": x_bf[b], "w": wpack} for b in range(B)]
    kwargs = {}
    if TRACE:
        _install_trace_hook()
        bass_utils.upload_artifacts = lambda tmpdir: "local://" + tmpdir
        kwargs["trace"] = True
    res = bass_utils.run_bass_kernel_spmd(
        nc, in_maps, core_ids=list(range(B)), **kwargs
    )
    LAST_RESULTS["exec_time_ns"] = res.exec_time_ns
    LAST_RESULTS["mean_exec_time_ns"] = res.mean_exec_time_ns

    out = np.empty((B, C * (NUM_KERNELS + 1), H, W), np.float32)
    out[:, :C] = x
    for b in range(B):
        yb = res.results[b]["y"]  # [63, 128, 2048] f16, partition-major
        out[b, C:] = (
            yb.astype(np.float32)
            .reshape(C * NUM_KERNELS, P, NBLK, W)
            .transpose(0, 2, 1, 3)
            .reshape(C * NUM_KERNELS, H, W)
        )
    return out


# revision 11
# speedup vs baseline: 1.2157x; 1.2157x over previous
"""Trainium2 Bass kernel v2: 21 depthwise Gaussian blurs + channel concat.

Problem: x (8, 3, 512, 512) f32 -> out (8, 66, 512, 512) f32 where
out = concat([x, blur_0(x), ..., blur_20(x)], axis=1) and blur_i is a
depthwise 2D Gaussian conv (reflect padding, kernel sizes 3..21).

Strategy (pure data parallel, 1 image per core across 8 cores):
  Each Gaussian is separable: conv2d(outer(g,g)) = conv_H(g) then conv_W(g).
  Each 1D conv (with reflect pad folded in) is a banded 512x512 matrix M.
  Per channel-image X (512x512):   Y_i = M_i @ X @ M_i^T
  Pass 1 computes Z^T = (M X)^T via out = lhsT.T @ rhs with lhsT = X-block
  (stationary) and rhs = band slabs of M^T (moving, ~130-148 cols per
  128-row block).  Pass 2 repeats the same structure on Z^T, which
  transposes back, yielding Y in natural [h, w] layout.

v2 over baseline:
  - output stored as f16 on device (33 MB vs 66 MB of HBM writes per
    core); host upcasts to f32.  Per-channel rel err stays ~4e-3
    (bf16 matmuls dominate), well under the 2e-2 gate.
  - passthrough channels (out[:, :3] = x) are filled on the host; the
    device only computes + writes the 63 blur channels.
  - x is cast to bf16 on the host; device loads bf16 directly (no
    GpSimd cast pass, half the input DMA).
  - PSUM evacuation copies rotate across ACT/DVE/Pool (5:4:3) instead
    of ACT/DVE only.
"""

import numpy as np
import ml_dtypes

B, C, H, W = 8, 3, 512, 512
N = 512
P = 128
NBLK = N // P  # 4

NUM_KERNELS = 21
MAX_KSIZE = 21
INIT_KSIZE = 3
_INCREMENT = (MAX_KSIZE - INIT_KSIZE) / (NUM_KERNELS - 2)
KSIZES = [
    min(MAX_KSIZE, int(INIT_KSIZE + i * _INCREMENT // 2 * 2))
    for i in range(NUM_KERNELS)
]
SIGMAS = np.linspace(0.5, 1.0, NUM_KERNELS)

import os as _os

TRACE = False  # set True (from a driver) to capture an NTFF profile
MERGE_LDW = _os.environ.get("KMERGE_LDW", "0") == "1"  # fold LDWEIGHTS into matmuls
LDW_OPT = _os.environ.get("KLDW_OPT", "0") == "1"  # --enable-ldw-opt=true (needs MERGE_LDW)
LAST_RESULTS = {}  # driver-inspectable: exec_time_ns etc.


def _gauss1d(k, sigma):
    # Matches reference _gauss_kernel numerics: float32 arange, float64 sigma
    # promotes the math to float64; normalized to sum 1.
    x = np.arange(k, dtype=np.float32)
    g = np.exp(-((x - k // 2) ** 2) / (2.0 * sigma**2))
    return g / g.sum()


def _conv_matrix(g, n=N):
    """Banded matrix M (float64) s.t. y = M @ x computes the reflect-padded
    1D convolution with taps g."""
    k = len(g)
    p = (k - 1) // 2
    M = np.zeros((n, n), np.float64)
    for r in range(n):
        for t in range(k):
            c = r + t - p
            if c < 0:
                c = -c
            elif c >= n:
                c = 2 * (n - 1) - c
            M[r, c] += g[t]
    return M


def _slab_geometry():
    """Per (kernel, block) slab column ranges in M^T, plus pack offsets."""
    geo = []  # [i][b] = (clo, chi, off)
    off = 0
    for i in range(NUM_KERNELS):
        p = (KSIZES[i] - 1) // 2
        row = []
        for b in range(NBLK):
            clo = max(0, P * b - p)
            chi = min(N, P * b + P + p)
            row.append((clo, chi, off))
            off += chi - clo
        geo.append(row)
    return geo, off


def _build_wpack():
    geo, totalw = _slab_geometry()
    wpack = np.zeros((P, totalw), ml_dtypes.bfloat16)
    for i in range(NUM_KERNELS):
        MT = _conv_matrix(_gauss1d(KSIZES[i], SIGMAS[i])).T
        for b in range(NBLK):
            clo, chi, off = geo[i][b]
            wpack[:, off : off + (chi - clo)] = MT[P * b : P * b + P, clo:chi].astype(
                ml_dtypes.bfloat16
            )
    return geo, totalw, wpack


_GEO, _TOTALW, _WPACK = None, None, None
_NC = None


def _consts():
    global _GEO, _TOTALW, _WPACK
    if _WPACK is None:
        _GEO, _TOTALW, _WPACK = _build_wpack()
    return _GEO, _TOTALW, _WPACK


def _build_nc():
    import concourse.bacc as bacc
    import concourse.mybir as mybir
    from concourse.tile import TileContext

    geo, totalw, _ = _consts()
    bf16 = mybir.dt.bfloat16
    f16 = mybir.dt.float16
    f32 = mybir.dt.float32

    nc = bacc.Bacc("TRN2", target_bir_lowering=False)
    # Partition-major layouts: x_perm[c, p, b, :] = x[c, 128*b + p, :] and
    # y_perm[c, p, b, :] = y[c, 128*b + p, :].  Each DMA then moves one
    # 4 KiB contiguous DRAM run per partition (128 descriptors) instead of
    # four 1 KiB runs (512 descriptors) -- descriptor generation on the
    # issuing sequencer was a serial bottleneck.
    x = nc.dram_tensor("x", [C, P, NBLK * N], bf16, kind="ExternalInput")
    w = nc.dram_tensor("w", [P, totalw], bf16, kind="ExternalInput")
    y = nc.dram_tensor("y", [C * NUM_KERNELS, P, NBLK * N], f16, kind="ExternalOutput")

    # PSUM evacuation: GPSIMD cannot touch PSUM on TRN2, so only ACT
    # (measured ~1.04us per [128,1024] copy) and DVE (~1.15us) qualify.
    # Pass-1 (z) evacuations gate pass 2 on the PE, so the two z copies of
    # an item always go to OPPOSITE engines (concurrent, minimal latency).
    # y copies are distributed to balance total busy time (~69:57).
    ny = 0

    def evac_z(nc_, dst, src, half):
        if half == 0:
            nc_.scalar.copy(dst, src)
        else:
            nc_.vector.tensor_copy(dst, src)

    def evac_y(nc_, dst, src):
        nonlocal ny
        # 11:9 ACT:DVE, Bresenham-interleaved so neither engine sees runs
        use_act = (ny * 11) // 20 != ((ny + 1) * 11) // 20
        ny += 1
        if use_act:
            nc_.scalar.copy(dst, src)
        else:
            nc_.vector.tensor_copy(dst, src)

    with TileContext(nc) as tc:
        with (
            tc.tile_pool(name="wsb", bufs=1) as wpool,
            tc.tile_pool(name="xsb", bufs=12) as xpool,
            tc.tile_pool(name="zt", bufs=8) as ztpool,
            tc.tile_pool(name="yo", bufs=6) as ypool,
            tc.tile_pool(name="ps1", bufs=2, space="PSUM") as ps1,
            tc.tile_pool(name="ps2", bufs=2, space="PSUM") as ps2,
        ):
            # One [128, 2048] load per channel; column range [512j, 512j+512)
            # of the tile is x rows [128j, 128j+128).
            xch = {}
            for ci in range(C):
                t = xpool.tile([P, NBLK * N], bf16, tag="x")
                nc.sync.dma_start(t[:], x[ci])
                xch[ci] = t

            # band slabs, chunked so early kernels start before the full load
            wsb = wpool.tile([P, totalw], bf16)
            bounds = [geo[i][0][2] for i in range(0, NUM_KERNELS, 3)] + [totalw]
            for a, b in zip(bounds[:-1], bounds[1:]):
                nc.sync.dma_start(wsb[:, a:b], w[:, a:b])

            def emit_pass1(ci, i):
                # ---- pass 1: Z^T[wb] = sum_j X[j,wb]^T @ slab(i,j) ----
                # two wb blocks share one 2-bank PSUM tile -> one big copy
                xt = xch[ci]
                zt = []
                for wb2 in range(NBLK // 2):
                    psz = ps1.tile([P, 2 * N], f32, tag="psz")
                    for half in range(2):
                        wb = 2 * wb2 + half
                        for j in range(NBLK):
                            clo, chi, off = geo[i][j]
                            nc.tensor.matmul(
                                psz[:, half * N + clo : half * N + chi],
                                xt[:, N * j + P * wb : N * j + P * wb + P],
                                wsb[:, off : off + (chi - clo)],
                                start=(j == 0),
                                stop=(j == NBLK - 1),
                            )
                    zt2 = ztpool.tile([P, 2 * N], bf16, tag="zt")
                    evac_z(nc, zt2[:], psz[:], wb2)
                    zt.append(zt2)
                return zt

            def emit_pass2(ci, i, zt):
                # ---- pass 2: Y[hb] = sum_wb Z^T[wb,hb]^T @ slab(i,wb) ----
                def ztap(wb, hb):
                    # Z^T[wb] block, columns for h-block hb
                    return zt[wb // 2][:, (wb % 2) * N + P * hb : (wb % 2) * N + P * hb + P]

                cout = C * i + ci
                yo = ypool.tile([P, 4 * N], f16, tag="yo")
                for hb2 in range(NBLK // 2):
                    psy = ps2.tile([P, 2 * N], f32, tag="psy")
                    for half in range(2):
                        hb = 2 * hb2 + half
                        for wb in range(NBLK):
                            clo, chi, off = geo[i][wb]
                            nc.tensor.matmul(
                                psy[:, half * N + clo : half * N + chi],
                                ztap(wb, hb),
                                wsb[:, off : off + (chi - clo)],
                                start=(wb == 0),
                                stop=(wb == NBLK - 1),
                            )
                    evac_y(nc, yo[:, hb2 * 2 * N : (hb2 + 1) * 2 * N], psy[:])
                nc.sync.dma_start(y[cout], yo[:])

            # Software-pipeline by one stage: emit pass 1 of item k+1 before
            # pass 2 of item k, so the in-order PE has independent matmuls to
            # run while item k's PSUM->SBUF evacuations are in flight.
            items = [(ci, i) for ci in range(C) for i in range(NUM_KERNELS)]
            zt_prev = emit_pass1(*items[0])
            for idx, (ci, i) in enumerate(items):
                zt_next = (
                    emit_pass1(*items[idx + 1]) if idx + 1 < len(items) else None
                )
                emit_pass2(ci, i, zt_prev)
                zt_prev = zt_next

    nc.finalize()
    if MERGE_LDW:
        _merge_ldweights(nc)
    return nc


def _merge_ldweights(nc):
    """Fold the Tile-emitted standalone InstLdweights back into their
    InstMatmult (self-loading form) so walrus's LDW optimization (fast
    weight load) can apply.  LDWs carrying sync waits are replaced by an
    InstEventSemaphore stub at the same position to preserve ordering."""
    import concourse.mybir as mybir

    ev = 0
    for blk in nc.m.functions[0].blocks:
        insts = blk.instructions
        new = []
        changed = False
        for ins in insts:
            tn = type(ins).__name__
            if tn == "InstLdweights":
                changed = True
                si = ins.sync_info
                if si is not None and (si.on_wait or si.on_update):
                    e = mybir.InstEventSemaphore(
                        name=f"ldw_ev_{ev}", ins=[], outs=[]
                    )
                    ev += 1
                    e.engine = ins.engine
                    e.sync_info = si
                    new.append(e)
                continue
            if tn == "InstMatmult":
                ins.ldweights = True
            new.append(ins)
        if changed:
            del insts[:]
            insts.extend(new)


def _patch_ldw_opt():
    import concourse.bass_utils as bass_utils

    if getattr(bass_utils, "_ldw_opt_patched", False):
        return
    orig = bass_utils.run_command

    def patched(argv, **kw):
        argv = [
            "--enable-ldw-opt=true" if a == "--enable-ldw-opt=false" else a
            for a in argv
        ]
        return orig(argv, **kw)

    bass_utils.run_command = patched
    bass_utils._ldw_opt_patched = True


def _get_nc():
    global _NC
    if _NC is None:
        _NC = _build_nc()
    return _NC


def _install_trace_hook():
    """Best-effort NTFF profiling hook for axon (used when TRACE=True)."""
    import sys
    import types

    if "antenv.axon_hooks" in sys.modules:
        return
    m = types.ModuleType("antenv.axon_hooks")
    m._hook = None
    m.set_axon_ntff_profile_hook = lambda h: setattr(m, "_hook", h)
    m.get_axon_ntff_profile_hook = lambda: m._hook
    sys.modules["antenv.axon_hooks"] = m
    try:
        import antenv

        antenv.axon_hooks = m
        from trn_agent_boot.trn_boot import _ntff_profile_via_ctypes

        m._hook = _ntff_profile_via_ctypes("/opt/axon/libaxon_pjrt.so")
    except Exception:
        pass


def kernel(x):
    import concourse.bass_utils as bass_utils

    if LDW_OPT:
        _patch_ldw_opt()
    x = np.asarray(x, dtype=np.float32)
    assert x.shape == (B, C, H, W), x.shape
    _, _, wpack = _consts()
    nc = _get_nc()

    # partition-major device layout: x_perm[c, p, b*512+w] = x[c, 128b+p, w]
    x_bf = np.ascontiguousarray(
        x.astype(ml_dtypes.bfloat16)
        .reshape(B, C, NBLK, P, W)
        .transpose(0, 1, 3, 2, 4)
        .reshape(B, C, P, NBLK * W)
    )
    in_maps = [{"x": x_bf[b], "w": wpack} for b in range(B)]
    kwargs = {}
    if TRACE:
        _install_trace_hook()
        bass_utils.upload_artifacts = lambda tmpdir: "local://" + tmpdir
        kwargs["trace"] = True
    res = bass_utils.run_bass_kernel_spmd(
        nc, in_maps, core_ids=list(range(B)), **kwargs
    )
    LAST_RESULTS["exec_time_ns"] = res.exec_time_ns
    LAST_RESULTS["mean_exec_time_ns"] = res.mean_exec_time_ns

    out = np.empty((B, C * (NUM_KERNELS + 1), H, W), np.float32)
    out[:, :C] = x
    for b in range(B):
        yb = res.results[b]["y"]  # [63, 128, 2048] f16, partition-major
        out[b, C:] = (
            yb.astype(np.float32)
            .reshape(C * NUM_KERNELS, P, NBLK, W)
            .transpose(0, 2, 1, 3)
            .reshape(C * NUM_KERNELS, H, W)
        )
    return out


# revision 13
# speedup vs baseline: 1.2321x; 1.0135x over previous
"""Trainium2 Bass kernel v2: 21 depthwise Gaussian blurs + channel concat.

Problem: x (8, 3, 512, 512) f32 -> out (8, 66, 512, 512) f32 where
out = concat([x, blur_0(x), ..., blur_20(x)], axis=1) and blur_i is a
depthwise 2D Gaussian conv (reflect padding, kernel sizes 3..21).

Strategy (pure data parallel, 1 image per core across 8 cores):
  Each Gaussian is separable: conv2d(outer(g,g)) = conv_H(g) then conv_W(g).
  Each 1D conv (with reflect pad folded in) is a banded 512x512 matrix M.
  Per channel-image X (512x512):   Y_i = M_i @ X @ M_i^T
  Pass 1 computes Z^T = (M X)^T via out = lhsT.T @ rhs with lhsT = X-block
  (stationary) and rhs = band slabs of M^T (moving, ~130-148 cols per
  128-row block).  Pass 2 repeats the same structure on Z^T, which
  transposes back, yielding Y in natural [h, w] layout.

v2 over baseline:
  - output stored as f16 on device (33 MB vs 66 MB of HBM writes per
    core); host upcasts to f32.  Per-channel rel err stays ~4e-3
    (bf16 matmuls dominate), well under the 2e-2 gate.
  - passthrough channels (out[:, :3] = x) are filled on the host; the
    device only computes + writes the 63 blur channels.
  - x is cast to bf16 on the host; device loads bf16 directly (no
    GpSimd cast pass, half the input DMA).
  - PSUM evacuation copies rotate across ACT/DVE/Pool (5:4:3) instead
    of ACT/DVE only.
"""

import numpy as np
import ml_dtypes

B, C, H, W = 8, 3, 512, 512
N = 512
P = 128
NBLK = N // P  # 4

NUM_KERNELS = 21
MAX_KSIZE = 21
INIT_KSIZE = 3
_INCREMENT = (MAX_KSIZE - INIT_KSIZE) / (NUM_KERNELS - 2)
KSIZES = [
    min(MAX_KSIZE, int(INIT_KSIZE + i * _INCREMENT // 2 * 2))
    for i in range(NUM_KERNELS)
]
SIGMAS = np.linspace(0.5, 1.0, NUM_KERNELS)

import os as _os

TRACE = False  # set True (from a driver) to capture an NTFF profile
MERGE_LDW = _os.environ.get("KMERGE_LDW", "0") == "1"  # fold LDWEIGHTS into matmuls
LDW_OPT = _os.environ.get("KLDW_OPT", "0") == "1"  # --enable-ldw-opt=true (needs MERGE_LDW)
LAST_RESULTS = {}  # driver-inspectable: exec_time_ns etc.


def _gauss1d(k, sigma):
    # Matches reference _gauss_kernel numerics: float32 arange, float64 sigma
    # promotes the math to float64; normalized to sum 1.
    x = np.arange(k, dtype=np.float32)
    g = np.exp(-((x - k // 2) ** 2) / (2.0 * sigma**2))
    return g / g.sum()


def _conv_matrix(g, n=N):
    """Banded matrix M (float64) s.t. y = M @ x computes the reflect-padded
    1D convolution with taps g."""
    k = len(g)
    p = (k - 1) // 2
    M = np.zeros((n, n), np.float64)
    for r in range(n):
        for t in range(k):
            c = r + t - p
            if c < 0:
                c = -c
            elif c >= n:
                c = 2 * (n - 1) - c
            M[r, c] += g[t]
    return M


def _slab_geometry():
    """Per (kernel, block) slab column ranges in M^T, plus pack offsets."""
    geo = []  # [i][b] = (clo, chi, off)
    off = 0
    for i in range(NUM_KERNELS):
        p = (KSIZES[i] - 1) // 2
        row = []
        for b in range(NBLK):
            clo = max(0, P * b - p)
            chi = min(N, P * b + P + p)
            row.append((clo, chi, off))
            off += chi - clo
        geo.append(row)
    return geo, off


def _build_wpack():
    geo, totalw = _slab_geometry()
    wpack = np.zeros((P, totalw), ml_dtypes.bfloat16)
    for i in range(NUM_KERNELS):
        MT = _conv_matrix(_gauss1d(KSIZES[i], SIGMAS[i])).T
        for b in range(NBLK):
            clo, chi, off = geo[i][b]
            wpack[:, off : off + (chi - clo)] = MT[P * b : P * b + P, clo:chi].astype(
                ml_dtypes.bfloat16
            )
    return geo, totalw, wpack


_GEO, _TOTALW, _WPACK = None, None, None
_NC = None


def _consts():
    global _GEO, _TOTALW, _WPACK
    if _WPACK is None:
        _GEO, _TOTALW, _WPACK = _build_wpack()
    return _GEO, _TOTALW, _WPACK


def _build_nc():
    import concourse.bacc as bacc
    import concourse.mybir as mybir
    from concourse.tile import TileContext

    geo, totalw, _ = _consts()
    bf16 = mybir.dt.bfloat16
    f16 = mybir.dt.float16
    f32 = mybir.dt.float32

    nc = bacc.Bacc("TRN2", target_bir_lowering=False)
    # Partition-major layouts: x_perm[c, p, b, :] = x[c, 128*b + p, :] and
    # y_perm[c, p, b, :] = y[c, 128*b + p, :].  Each DMA then moves one
    # 4 KiB contiguous DRAM run per partition (128 descriptors) instead of
    # four 1 KiB runs (512 descriptors) -- descriptor generation on the
    # issuing sequencer was a serial bottleneck.
    x = nc.dram_tensor("x", [C, P, NBLK * N], bf16, kind="ExternalInput")
    w = nc.dram_tensor("w", [P, totalw], bf16, kind="ExternalInput")
    y = nc.dram_tensor("y", [C * NUM_KERNELS, P, NBLK * N], f16, kind="ExternalOutput")

    # PSUM evacuation engine rotation.  GPSIMD cannot touch PSUM on TRN2,
    # so only ACT (measured ~1.04us per [128,1024] copy) and DVE (~1.15us)
    # qualify; 9:8 interleave balances busy time.
    evac_cycle = ["a", "d"] * 8 + ["a"]
    ncopy = 0

    def evac(nc_, dst, src):
        nonlocal ncopy
        e = evac_cycle[ncopy % len(evac_cycle)]
        ncopy += 1
        if e == "a":
            nc_.scalar.copy(dst, src)
        else:
            nc_.vector.tensor_copy(dst, src)

    evac_z = lambda nc_, dst, src, half: evac(nc_, dst, src)
    evac_y = evac

    with TileContext(nc) as tc:
        with (
            tc.tile_pool(name="wsb", bufs=1) as wpool,
            tc.tile_pool(name="xsb", bufs=12) as xpool,
            tc.tile_pool(name="zt", bufs=6) as ztpool,
            tc.tile_pool(name="yo", bufs=4) as ypool,
            tc.tile_pool(name="ps1", bufs=2, space="PSUM") as ps1,
            tc.tile_pool(name="ps2", bufs=2, space="PSUM") as ps2,
        ):
            # One [128, 2048] load per channel; column range [512j, 512j+512)
            # of the tile is x rows [128j, 128j+128).
            xch = {}
            for ci in range(C):
                t = xpool.tile([P, NBLK * N], bf16, tag="x")
                nc.sync.dma_start(t[:], x[ci])
                xch[ci] = t

            # band slabs, chunked so early kernels start before the full load
            wsb = wpool.tile([P, totalw], bf16)
            bounds = [geo[i][0][2] for i in range(0, NUM_KERNELS, 3)] + [totalw]
            for a, b in zip(bounds[:-1], bounds[1:]):
                nc.sync.dma_start(wsb[:, a:b], w[:, a:b])

            def emit_pass1(ci, i):
                # ---- pass 1: Z^T[wb] = sum_j X[j,wb]^T @ slab(i,j) ----
                # two wb blocks share one 2-bank PSUM tile -> one big copy
                xt = xch[ci]
                zt = []
                for wb2 in range(NBLK // 2):
                    psz = ps1.tile([P, 2 * N], f32, tag="psz")
                    for half in range(2):
                        wb = 2 * wb2 + half
                        for j in range(NBLK):
                            clo, chi, off = geo[i][j]
                            nc.tensor.matmul(
                                psz[:, half * N + clo : half * N + chi],
                                xt[:, N * j + P * wb : N * j + P * wb + P],
                                wsb[:, off : off + (chi - clo)],
                                start=(j == 0),
                                stop=(j == NBLK - 1),
                            )
                    zt2 = ztpool.tile([P, 2 * N], bf16, tag="zt")
                    evac_z(nc, zt2[:], psz[:], wb2)
                    zt.append(zt2)
                return zt

            def emit_pass2(ci, i, zt):
                # ---- pass 2: Y[hb] = sum_wb Z^T[wb,hb]^T @ slab(i,wb) ----
                def ztap(wb, hb):
                    # Z^T[wb] block, columns for h-block hb
                    return zt[wb // 2][:, (wb % 2) * N + P * hb : (wb % 2) * N + P * hb + P]

                cout = C * i + ci
                yo = ypool.tile([P, 4 * N], f16, tag="yo")
                for hb2 in range(NBLK // 2):
                    psy = ps2.tile([P, 2 * N], f32, tag="psy")
                    for half in range(2):
                        hb = 2 * hb2 + half
                        for wb in range(NBLK):
                            clo, chi, off = geo[i][wb]
                            nc.tensor.matmul(
                                psy[:, half * N + clo : half * N + chi],
                                ztap(wb, hb),
                                wsb[:, off : off + (chi - clo)],
                                start=(wb == 0),
                                stop=(wb == NBLK - 1),
                            )
                    evac_y(nc, yo[:, hb2 * 2 * N : (hb2 + 1) * 2 * N], psy[:])
                nc.sync.dma_start(y[cout], yo[:])

            # Software-pipeline by one stage: emit pass 1 of item k+1 before
            # pass 2 of item k, so the in-order PE has independent matmuls to
            # run while item k's PSUM->SBUF evacuations are in flight.
            items = [(ci, i) for ci in range(C) for i in range(NUM_KERNELS)]
            zt_prev = emit_pass1(*items[0])
            for idx, (ci, i) in enumerate(items):
                zt_next = (
                    emit_pass1(*items[idx + 1]) if idx + 1 < len(items) else None
                )
                emit_pass2(ci, i, zt_prev)
                zt_prev = zt_next

    nc.finalize()
    if MERGE_LDW:
        _merge_ldweights(nc)
    return nc


def _merge_ldweights(nc):
    """Fold the Tile-emitted standalone InstLdweights back into their
    InstMatmult (self-loading form) so walrus's LDW optimization (fast
    weight load) can apply.  LDWs carrying sync waits are replaced by an
    InstEventSemaphore stub at the same position to preserve ordering."""
    import concourse.mybir as mybir

    ev = 0
    for blk in nc.m.functions[0].blocks:
        insts = blk.instructions
        new = []
        changed = False
        for ins in insts:
            tn = type(ins).__name__
            if tn == "InstLdweights":
                changed = True
                si = ins.sync_info
                if si is not None and (si.on_wait or si.on_update):
                    e = mybir.InstEventSemaphore(
                        name=f"ldw_ev_{ev}", ins=[], outs=[]
                    )
                    ev += 1
                    e.engine = ins.engine
                    e.sync_info = si
                    new.append(e)
                continue
            if tn == "InstMatmult":
                ins.ldweights = True
            new.append(ins)
        if changed:
            del insts[:]
            insts.extend(new)


def _patch_ldw_opt():
    import concourse.bass_utils as bass_utils

    if getattr(bass_utils, "_ldw_opt_patched", False):
        return
    orig = bass_utils.run_command

    def patched(argv, **kw):
        argv = [
            "--enable-ldw-opt=true" if a == "--enable-ldw-opt=false" else a
            for a in argv
        ]
        return orig(argv, **kw)

    bass_utils.run_command = patched
    bass_utils._ldw_opt_patched = True


def _get_nc():
    global _NC
    if _NC is None:
        _NC = _build_nc()
    return _NC


def _install_trace_hook():
    """Best-effort NTFF profiling hook for axon (used when TRACE=True)."""
    import sys
    import types

    if "antenv.axon_hooks" in sys.modules:
        return
    m = types.ModuleType("antenv.axon_hooks")
    m._hook = None
    m.set_axon_ntff_profile_hook = lambda h: setattr(m, "_hook", h)
    m.get_axon_ntff_profile_hook = lambda: m._hook
    sys.modules["antenv.axon_hooks"] = m
    try:
        import antenv

        antenv.axon_hooks = m
        from trn_agent_boot.trn_boot import _ntff_profile_via_ctypes

        m._hook = _ntff_profile_via_ctypes("/opt/axon/libaxon_pjrt.so")
    except Exception:
        pass


def kernel(x):
    import concourse.bass_utils as bass_utils

    if LDW_OPT:
        _patch_ldw_opt()
    x = np.asarray(x, dtype=np.float32)
    assert x.shape == (B, C, H, W), x.shape
    _, _, wpack = _consts()
    nc = _get_nc()

    # partition-major device layout: x_perm[c, p, b*512+w] = x[c, 128b+p, w]
    x_bf = np.ascontiguousarray(
        x.astype(ml_dtypes.bfloat16)
        .reshape(B, C, NBLK, P, W)
        .transpose(0, 1, 3, 2, 4)
        .reshape(B, C, P, NBLK * W)
    )
    in_maps = [{"x": x_bf[b], "w": wpack} for b in range(B)]
    kwargs = {}
    if TRACE:
        _install_trace_hook()
        bass_utils.upload_artifacts = lambda tmpdir: "local://" + tmpdir
        kwargs["trace"] = True
    res = bass_utils.run_bass_kernel_spmd(
        nc, in_maps, core_ids=list(range(B)), **kwargs
    )
    LAST_RESULTS["exec_time_ns"] = res.exec_time_ns
    LAST_RESULTS["mean_exec_time_ns"] = res.mean_exec_time_ns

    out = np.empty((B, C * (NUM_KERNELS + 1), H, W), np.float32)
    out[:, :C] = x
    for b in range(B):
        yb = res.results[b]["y"]  # [63, 128, 2048] f16, partition-major
        out[b, C:] = (
            yb.astype(np.float32)
            .reshape(C * NUM_KERNELS, P, NBLK, W)
            .transpose(0, 2, 1, 3)
            .reshape(C * NUM_KERNELS, H, W)
        )
    return out
